# revision 5
# baseline (speedup 1.0000x reference)
"""v2: Causal single-head attention (B=4, S=4096, D=1024, H=64) on 8 TRN2 cores.

Same 8-core split as v1 (4 batches x 2 interleaved query-fold roles, 512-row
chunks), with:
- bf16 inputs end-to-end (halves strip DMA vs f32)
- col-tiled projection pairs: two strips' projections run concurrently in
  the two column halves of the PE array (out psum partitions 0:64 / 64:128)
- row-packed score pairs: even/odd key blocks' QK^T run concurrently in the
  two row halves (kt/qt rows 64:128 duplicate 0:64)
- one exp per 2 key blocks ([128, 2, 512] PSUM tile), mask applied after,
  diag pairs trimmed to the causal width
- single merged K+V pairwise AllGather (bf16) per pass, software-pipelined
  one pass ahead: pass i projects K/V for pass i+1 and issues its collective,
  so each pass starts with peer K/V already in flight/complete
- per-pass tiles double-buffered by parity; collective-dependent DMAs ride
  the scalar HWDGE ring so they never stall the strip-load FIFO on sync

Softmax denominator from an ones-column appended to V (PV lhsT M=65);
division on host: output oT [65, 2048] bf16 = [numerator^T; denominator].
"""

import numpy as np
import ml_dtypes

import concourse.bacc as bacc
import concourse.mybir as mybir
import concourse.tile as tile
from concourse.masks import make_identity
from concourse.bass_utils import run_bass_kernel_spmd

B, S, D, H = 4, 4096, 1024, 64
SBLK = 512
NCH = D // 128      # 8 contraction chunks
QLOC = 2048
NSLOT = QLOC // SBLK  # 4
NVB = QLOC // 128     # 16 v blocks per fold

F32 = mybir.dt.float32
BF16 = mybir.dt.bfloat16

RG_PAIRS = [[0, 1], [2, 3], [4, 5], [6, 7]]


def build_kernel(repeat: int = 1, phase: str = "full", rowpack: bool = True,
                 ptbufs: int = 8, ppbufs: int = 1, pobufs: int = 3):
    nc = bacc.Bacc("TRN2", target_bir_lowering=False, debug=False, num_devices=8)

    qT = nc.dram_tensor("qT", [D, QLOC], BF16, kind="ExternalInput")
    kT = nc.dram_tensor("kT", [D, QLOC], BF16, kind="ExternalInput")
    vT = nc.dram_tensor("vT", [D, QLOC], BF16, kind="ExternalInput")
    wqT = nc.dram_tensor("wqT", [D, H], BF16, kind="ExternalInput")
    wkT = nc.dram_tensor("wkT", [D, H], BF16, kind="ExternalInput")
    wvT = nc.dram_tensor("wvT", [D, H], BF16, kind="ExternalInput")
    masks = nc.dram_tensor("masks", [4, 128, SBLK], BF16, kind="ExternalInput")
    rolesel = nc.dram_tensor("rolesel", [128, 2], F32, kind="ExternalInput")
    out = nc.dram_tensor("oT", [H + 1, QLOC], BF16, kind="ExternalOutput")

    with tile.TileContext(nc) as tc:
        with (
            tc.tile_pool(name="const", bufs=1) as const_pool,
            tc.tile_pool(name="big", bufs=1) as big_pool,
            tc.tile_pool(name="strips", bufs=4) as strip_pool,
            tc.tile_pool(name="work", bufs=2) as work_pool,
            tc.tile_pool(name="pp", bufs=ppbufs, space="PSUM") as pp,
            tc.tile_pool(name="ps_sc", bufs=2, space="PSUM") as ps_sc,
            tc.tile_pool(name="ps_o", bufs=pobufs, space="PSUM") as ps_o,
            tc.tile_pool(name="dram", bufs=1, space="DRAM") as dram_pool,
        ):
            # ---- constants ----
            wq_sb = const_pool.tile([128, NCH, H], BF16)
            wk_sb = const_pool.tile([128, NCH, H], BF16)
            wv_sb = const_pool.tile([128, NCH, H], BF16)
            nc.sync.dma_start(wq_sb[:], wqT.rearrange("(c p) h -> p c h", p=128))
            nc.sync.dma_start(wk_sb[:], wkT.rearrange("(c p) h -> p c h", p=128))
            nc.sync.dma_start(wv_sb[:], wvT.rearrange("(c p) h -> p c h", p=128))
            mask_sb = const_pool.tile([128, 4, SBLK], BF16)
            nc.sync.dma_start(mask_sb[:], masks.rearrange("m p q -> p m q"))
            rs = const_pool.tile([128, 2], F32)
            nc.sync.dma_start(rs[:], rolesel[:])
            ident = const_pool.tile([128, 128], BF16)
            make_identity(nc, ident[:])

            # ---- per-pass tensors, double-buffered by pass parity ----
            ot_sb = big_pool.tile([H + 1, QLOC], BF16)
            qt_bufs = [
                big_pool.tile([128, QLOC], BF16, name=f"qt{i}") for i in range(2)
            ]
            ktl_bufs = [
                big_pool.tile([128, QLOC], BF16, name=f"ktl{i}") for i in range(2)
            ]
            ktp_bufs = [
                big_pool.tile([128, QLOC], BF16, name=f"ktp{i}") for i in range(2)
            ]
            vloc_bufs = [
                big_pool.tile([128, NVB, H + 1], BF16, name=f"vloc{i}")
                for i in range(2)
            ]
            vpa_bufs = [
                big_pool.tile([128, NVB, H + 1], BF16, name=f"vpa{i}")
                for i in range(2)
            ]
            vpd_bufs = [
                big_pool.tile([128, NVB, H + 1], BF16, name=f"vpd{i}")
                for i in range(2)
            ]
            ksc_bufs = [
                (
                    big_pool.tile([H, QLOC], BF16, name=f"ksc0_{i}"),
                    big_pool.tile([H, QLOC], BF16, name=f"ksc1_{i}"),
                    big_pool.tile([128, NVB, H], BF16, name=f"vsc0_{i}"),
                    big_pool.tile([128, NVB, H], BF16, name=f"vsc1_{i}"),
                )
                for i in range(2)
            ]

            const_pt = const_pool.tile([128, 2, SBLK], BF16)
            nc.vector.memset(const_pt[:], 0.001)
            for vb in vloc_bufs + vpa_bufs:
                nc.vector.memset(vb[:, :, H], 1.0)

            def load_strip(src_dram, s_off):
                strip = strip_pool.tile([128, NCH, SBLK], BF16, tag="xstrip")
                nc.sync.dma_start(
                    strip[:],
                    src_dram[:, s_off : s_off + SBLK].rearrange(
                        "(c p) s -> p c s", p=128
                    ),
                )
                return strip

            def proj_pair_mm(w_sb, sa, sb2):
                ps = pp.tile([128, SBLK], F32, tag="proj")
                for c in range(NCH):
                    nc.tensor.matmul(
                        ps[0:64, :], w_sb[:, c, :], sa[:, c, :],
                        start=(c == 0), stop=(c == NCH - 1),
                        skip_group_check=True,
                    )
                    nc.tensor.matmul(
                        ps[64:128, :], w_sb[:, c, :], sb2[:, c, :],
                        start=(c == 0), stop=(c == NCH - 1),
                        skip_group_check=True,
                    )
                return ps

            def project_qk(dst, w_sb, src_dram, t):
                """Strip pair (2t, 2t+1) -> dst rows 0:64 / 64:128 + dup."""
                a, b = 2 * t, 2 * t + 1
                sa = load_strip(src_dram, a * SBLK)
                sb2 = load_strip(src_dram, b * SBLK)
                ps = proj_pair_mm(w_sb, sa, sb2)
                nc.vector.tensor_copy(
                    dst[0:64, a * SBLK : (a + 1) * SBLK], ps[0:64, :]
                )
                nc.vector.tensor_copy(
                    dst[64:128, b * SBLK : (b + 1) * SBLK], ps[64:128, :]
                )
                nc.sync.dma_start(
                    dst[64:128, a * SBLK : (a + 1) * SBLK],
                    dst[0:64, a * SBLK : (a + 1) * SBLK],
                )
                nc.sync.dma_start(
                    dst[0:64, b * SBLK : (b + 1) * SBLK],
                    dst[64:128, b * SBLK : (b + 1) * SBLK],
                )

            def project_v(dst_vloc, t):
                a, b = 2 * t, 2 * t + 1
                sa = load_strip(vT, a * SBLK)
                sb2 = load_strip(vT, b * SBLK)
                ps = proj_pair_mm(wv_sb, sa, sb2)
                vt_stage = work_pool.tile([128, SBLK], BF16, tag="vt_stage")
                nc.vector.tensor_copy(vt_stage[:], ps[:])
                for i in range(4):
                    for base, g in ((0, a), (64, b)):
                        tp = pp.tile([128, H], BF16, tag="proj", name="tp")
                        nc.tensor.transpose(
                            tp[:],
                            vt_stage[base : base + 64, i * 128 : (i + 1) * 128],
                            ident[base : base + 64, base : base + 64],
                        )
                        nc.vector.tensor_copy(dst_vloc[:, g * 4 + i, :H], tp[:])

            def project_kv_and_cc(nxt):
                """Project K/V for pass parity nxt, stage, issue collective."""
                for t in range(NSLOT // 2):
                    project_qk(ktl_bufs[nxt], wk_sb, kT, t)
                for t in range(NSLOT // 2):
                    project_v(vloc_bufs[nxt], t)
                kv_in = dram_pool.tile([128, 2048], BF16, tag=f"kv_in{nxt}")
                kv_out = dram_pool.tile([2, 128, 2048], BF16, tag=f"kv_out{nxt}")
                nc.gpsimd.dma_start(
                    kv_in[:, 0:1024].rearrange("(a p) c -> p a c", a=2),
                    ktl_bufs[nxt][0:64, :].rearrange("p (a c) -> p a c", a=2),
                )
                nc.gpsimd.dma_start(
                    kv_in[:, 1024:2048].rearrange("p (b h) -> p b h", b=NVB),
                    vloc_bufs[nxt][:, :, :H],
                )
                nc.gpsimd.collective_compute(
                    "AllGather", mybir.AluOpType.bypass,
                    replica_groups=RG_PAIRS,
                    ins=[kv_in.opt()], outs=[kv_out.opt()],
                )
                return kv_out

            def combine(par, kv_out):
                """Role-select peer K/V from the gathered pair."""
                kt_peer, vpeer_aug, vpeer_diag = (
                    ktp_bufs[par], vpa_bufs[par], vpd_bufs[par],
                )
                k_sc0, k_sc1, v_sc0, v_sc1 = ksc_bufs[par]
                nc.gpsimd.dma_start(
                    k_sc0[:].rearrange("p (a c) -> p a c", a=2),
                    kv_out[0][:, 0:1024].rearrange("(a p) c -> p a c", a=2),
                )
                nc.gpsimd.dma_start(
                    k_sc1[:].rearrange("p (a c) -> p a c", a=2),
                    kv_out[1][:, 0:1024].rearrange("(a p) c -> p a c", a=2),
                )
                ktmp = work_pool.tile([H, QLOC], BF16, tag="ktmp")
                for kc in range(2):
                    sl = slice(kc * QLOC // 2, (kc + 1) * QLOC // 2)
                    nc.vector.tensor_scalar_mul(
                        kt_peer[:H, sl], k_sc0[:, sl], rs[:H, 0:1]
                    )
                    nc.vector.tensor_scalar_mul(
                        ktmp[:, sl], k_sc1[:, sl], rs[:H, 1:2]
                    )
                    nc.vector.tensor_add(
                        kt_peer[:H, sl], kt_peer[:H, sl], ktmp[:, sl]
                    )
                    nc.gpsimd.dma_start(kt_peer[64:128, sl], kt_peer[0:64, sl])
                nc.gpsimd.dma_start(
                    v_sc0[:],
                    kv_out[0][:, 1024:2048].rearrange("p (b h) -> p b h", b=NVB),
                )
                nc.gpsimd.dma_start(
                    v_sc1[:],
                    kv_out[1][:, 1024:2048].rearrange("p (b h) -> p b h", b=NVB),
                )
                vtmp = work_pool.tile([128, NVB, H], BF16, tag="vtmp")
                nc.vector.tensor_scalar_mul(
                    vpeer_aug[:, :, :H], v_sc0[:], rs[:, 0:1]
                )
                nc.vector.tensor_scalar_mul(vtmp[:], v_sc1[:], rs[:, 1:2])
                nc.vector.tensor_add(
                    vpeer_aug[:, :, :H], vpeer_aug[:, :, :H], vtmp[:]
                )
                nc.vector.tensor_scalar_mul(vpeer_diag[:], vpeer_aug[:], rs[:, 0:1])

            def attn_pass(par):
                qt_sb = qt_bufs[par]
                kt_loc, kt_peer = ktl_bufs[par], ktp_bufs[par]
                vloc_aug, vpeer_aug, vpeer_diag = (
                    vloc_bufs[par], vpa_bufs[par], vpd_bufs[par],
                )

                def attn_pair(po, kt, p, va, s, mask2, first, last, trim=0):
                    w = SBLK - trim
                    ps2 = ps_sc.tile([128, 2, SBLK], F32, tag="scores")
                    j0, j1 = 2 * p, 2 * p + 1
                    qs = slice(s * SBLK + trim, (s + 1) * SBLK)
                    nc.tensor.matmul(
                        ps2[:, 0, :w], kt[0:64, j0 * 128 : (j0 + 1) * 128],
                        qt_sb[0:64, qs], start=True, stop=True,
                    )
                    if rowpack:
                        nc.tensor.matmul(
                            ps2[:, 1, :w], kt[64:128, j1 * 128 : (j1 + 1) * 128],
                            qt_sb[64:128, qs], start=True, stop=True,
                        )
                    else:
                        nc.tensor.matmul(
                            ps2[:, 1, :w], kt[0:64, j1 * 128 : (j1 + 1) * 128],
                            qt_sb[0:64, qs], start=True, stop=True,
                        )
                    if phase == "noexp":
                        pt2 = const_pt
                    else:
                        pt2 = work_pool.tile(
                            [128, 2, SBLK], BF16, tag="pt", bufs=ptbufs
                        )
                        nc.scalar.activation(
                            pt2[:, :, :w], ps2[:, :, :w],
                            mybir.ActivationFunctionType.Exp,
                        )
                        if mask2 is not None:
                            nc.vector.tensor_mul(
                                pt2[:, :, :w], pt2[:, :, :w], mask2
                            )
                    nc.tensor.matmul(
                        po[:, trim:], va[:, j0, :], pt2[:, 0, :w],
                        start=first, stop=False,
                    )
                    nc.tensor.matmul(
                        po[:, trim:], va[:, j1, :], pt2[:, 1, :w],
                        start=False, stop=last,
                    )

                def same_fold(s, po):
                    for g in range(s + 1):
                        for h2 in range(2):
                            trim = 256 * h2 if g == s else 0
                            mask2 = (
                                mask_sb[:, 2 * h2 : 2 * h2 + 2, trim:]
                                if g == s
                                else None
                            )
                            attn_pair(
                                po, kt_loc, 2 * g + h2, vloc_aug, s, mask2,
                                first=(g == 0 and h2 == 0), last=False,
                                trim=trim,
                            )

                def cross_fold(s, po):
                    for g in range(s + 1):
                        va = vpeer_diag if g == s else vpeer_aug
                        for h2 in range(2):
                            attn_pair(
                                po, kt_peer, 2 * g + h2, va, s, None,
                                first=False, last=(g == s and h2 == 1),
                            )

                pos = {}
                for s in (1, 2, 3) if pobufs >= 3 else (2, 3):
                    po = ps_o.tile([H + 1, SBLK], F32, tag="oT")
                    pos[s] = po
                    same_fold(s, po)
                for s in (1, 0, 2, 3) if pobufs >= 3 else (2, 3, 0, 1):
                    if s not in pos:
                        po = ps_o.tile([H + 1, SBLK], F32, tag="oT")
                        pos[s] = po
                        same_fold(s, po)
                    cross_fold(s, pos[s])
                    nc.vector.tensor_copy(
                        ot_sb[:, s * SBLK : (s + 1) * SBLK], pos[s][:]
                    )
                    nc.scalar.dma_start(
                        out[:, s * SBLK : (s + 1) * SBLK],
                        ot_sb[:, s * SBLK : (s + 1) * SBLK],
                    )

            if phase == "dma":
                for _rep in range(repeat):
                    for g in range(NSLOT):
                        for src in (kT, vT, qT):
                            strip = load_strip(src, g * SBLK)
                            nc.vector.tensor_copy(ot_sb[:1, :4], strip[:1, 0, :4])
                    nc.vector.memset(ot_sb[:], 0.0)
                    nc.sync.dma_start(out[:], ot_sb[:])
            else:
                # software pipeline: prologue projects K/V for pass 0 and
                # issues its collective; pass i then projects Q(i), combines,
                # projects K/V(i+1) + issues its collective, runs attention(i).
                kv_outs = {0: project_kv_and_cc(0)}
                for i in range(repeat):
                    par, nxt = i % 2, (i + 1) % 2
                    for t in range(NSLOT // 2):
                        project_qk(qt_bufs[par], wq_sb, qT, t)
                    combine(par, kv_outs[par])
                    if i + 1 < repeat:
                        kv_outs[nxt] = project_kv_and_cc(nxt)
                    attn_pass(par)

    nc.compile()
    return nc


def fold_rows(r):
    return np.concatenate(
        [np.arange(512 * (2 * s + r), 512 * (2 * s + r) + 512) for s in range(4)]
    )


def make_in_maps(q, k, v, Wq, Wk, Wv, mode=None):
    scale = 1.0 / np.sqrt(np.float32(H))
    wqT = np.ascontiguousarray((Wq * scale).T).astype(ml_dtypes.bfloat16)
    wkT = np.ascontiguousarray(Wk.T).astype(ml_dtypes.bfloat16)
    wvT = np.ascontiguousarray(Wv.T).astype(ml_dtypes.bfloat16)

    kk = np.arange(128)[:, None]
    qq = np.arange(SBLK)[None, :]
    masks = np.stack(
        [(qq >= kk + 128 * m).astype(ml_dtypes.bfloat16) for m in range(4)]
    )

    in_maps = []
    for c in range(8):
        b, r = c // 2, c % 2
        rows = fold_rows(r)
        rsel = np.zeros((128, 2), dtype=np.float32)
        rsel[:, 0] = 1.0 if r == 1 else 0.0
        rsel[:, 1] = 1.0 if r == 0 else 0.0
        in_maps.append(
            {
                "qT": np.ascontiguousarray(q[b][rows].T).astype(ml_dtypes.bfloat16),
                "kT": np.ascontiguousarray(k[b][rows].T).astype(ml_dtypes.bfloat16),
                "vT": np.ascontiguousarray(v[b][rows].T).astype(ml_dtypes.bfloat16),
                "wqT": wqT,
                "wkT": wkT,
                "wvT": wvT,
                "masks": masks,
                "rolesel": rsel,
            }
        )
    return in_maps


def assemble_output(results):
    out = np.zeros((B, S, H), dtype=np.float32)
    for c in range(8):
        b, r = c // 2, c % 2
        oT = np.asarray(results[c]["oT"], dtype=np.float32)
        for s in range(4):
            num = oT[:H, s * SBLK : (s + 1) * SBLK]
            den = oT[H, s * SBLK : (s + 1) * SBLK]
            g = 512 * (2 * s + r)
            out[b, g : g + 512, :] = (num / den[None, :]).T
    return out


_NC_CACHE = {}


def kernel(q, k, v, Wq, Wk, Wv):
    q = np.asarray(q, dtype=np.float32)
    k = np.asarray(k, dtype=np.float32)
    v = np.asarray(v, dtype=np.float32)
    Wq = np.asarray(Wq, dtype=np.float32)
    Wk = np.asarray(Wk, dtype=np.float32)
    Wv = np.asarray(Wv, dtype=np.float32)

    if "nc" not in _NC_CACHE:
        _NC_CACHE["nc"] = build_kernel()
    nc = _NC_CACHE["nc"]
    in_maps = make_in_maps(q, k, v, Wq, Wk, Wv)
    last_exc = None
    for attempt in range(3):
        try:
            res = run_bass_kernel_spmd(nc, in_maps, core_ids=list(range(8)))
            return assemble_output(res.results)
        except Exception as e:
            last_exc = e
            import time as _time

            _time.sleep(15 * (attempt + 1))
    raise last_exc
